# revision 1
# baseline (speedup 1.0000x reference)
"""Trainium2 Bass kernel for a 4-layer GNN-style MLP (ChebConv K=1) with
training-mode BatchNorm, global_add_pool over 64 graphs, and a 3-layer FC head.

Strategy (8 NeuronCores, data-parallel over nodes):
  - 12500 nodes/core, feature-major layout [feat_part(128) x nodes_free] so the
    whole matmul chain needs no transposes.
  - BatchNorm (batch statistics over all 100k nodes) is folded into the next
    matmul's weights: bn(h) @ w + b == h @ (s*w) + (t@w + b).  Per-feature
    sum/sumsq are accumulated on-chip and combined across cores with small
    AllReduces.
  - Engine balance per node-tile keeps the PE free of drain stalls (which
    would hold the PE at its low p-state clock): ACT drains 3 of 4 relu
    chunks (with sum accumulators), DVE drains 1 chunk and computes sumsq
    via tensor_tensor_reduce on the bf16 activations.
  - L0 (128->512) and L1 (512->512) are fused into one software-pipelined
    tile loop (L0 of tile j runs while L1 of tile j-1 drains).
  - Pooling: per-tile node sums come free from the relu accumulators;
    graph-boundary suffixes are corrected with per-tile 0/1 masks
    (tensor_tensor_reduce), then a tiny one-hot matmul scatters tile sums
    into the 64 graph bins; bn3's affine is applied post-AllReduce.
"""

import contextlib

import numpy as np

import concourse.bass as bass
import concourse.tile as tile
from concourse import bacc, mybir
from concourse import bass_utils

F32 = mybir.dt.float32
BF16 = mybir.dt.bfloat16

# Problem constants (hardcoded per contract).
N = 100000          # nodes
IN = 128            # input features
D = 512             # hidden dim
G = 64              # graphs
C = 10              # classes
EPS = 1e-5
NCORES = 8
NS = N // NCORES    # nodes per core = 12500
NT = 500            # node tile (free dim per matmul)
NTILES = NS // NT   # 25
KC = D // 128       # 4 chunks of the hidden dim
FN = float(N)
XSP = 1250          # x streaming span
NXSP = NS // XSP    # 5

AR_GROUPS = [list(range(NCORES))]
KMIX = 1  # mixed (multi-graph) tiles per core; set by build_program

import os
_V = set(os.environ.get("KERNEL_V", "").split(",")) - {""}
# tensor_tensor_reduce hangs TRN2 hardware in this codegen path (bisected
# 2026-08-08); default to the scalar_tensor_tensor fallback.
USE_TTR = "ttr" in _V
FUSE_P1 = "nofuse" not in _V     # pipeline L0+L1 in one tile loop
ALT_DMA_Q = "syncdma" not in _V  # consts on gpsimd DMA queue
POOL_STT = False  # walrus rejects TensorScalarPtr on the Pool engine

Relu = mybir.ActivationFunctionType.Relu
Copy = mybir.ActivationFunctionType.Copy
Sqrt = mybir.ActivationFunctionType.Sqrt
Sig = mybir.ActivationFunctionType.Sigmoid
ADD = mybir.AluOpType.add
MULT = mybir.AluOpType.mult
MAX = mybir.AluOpType.max
AXX = mybir.AxisListType.X


def _bcast_part(ap, nparts):
    """Stride-0 partition broadcast of a DRAM AP: [a, b] -> [nparts, a, b]."""
    return bass.AP(tensor=ap.tensor, offset=ap.offset,
                   ap=[[0, nparts]] + list(ap.ap))


def _build_host_inputs(inputs):
    """Shard + reshape the full problem inputs into per-core input maps.

    Nodes are permuted so that all but K tiles per core hold nodes of a
    single graph (pure tiles -> one-hot row in `arm`); the graph-remainder
    nodes are packed into K "mixed" tiles per core, pooled on-device via a
    node-level one-hot matmul (`oh`).  This removes the suffix-mask pass.
    """
    x = np.asarray(inputs["x"], np.float32)
    batch = np.asarray(inputs["batch"]).astype(np.int64)

    counts = np.bincount(batch, minlength=G).astype(np.float32).reshape(1, G)

    # node indices per graph (batch is sorted)
    starts = np.searchsorted(batch, np.arange(G + 1))
    pure_tiles = []   # (graph, node_index_array)
    rem_idx = []
    for g in range(G):
        idx = np.arange(starts[g], starts[g + 1])
        nfull = len(idx) // NT
        for t in range(nfull):
            pure_tiles.append((g, idx[t * NT:(t + 1) * NT]))
        rem_idx.append(idx[nfull * NT:])
    rem = np.concatenate(rem_idx)
    assert len(rem) % NT == 0
    mixed_tiles = [rem[i * NT:(i + 1) * NT] for i in range(len(rem) // NT)]
    M = len(mixed_tiles)
    K = (M + NCORES - 1) // NCORES
    # reclassify pure tiles as mixed to give every core exactly K mixed
    while len(mixed_tiles) < K * NCORES:
        g, idx = pure_tiles.pop()
        mixed_tiles.append(idx)
    npure = NTILES - K

    def chunk_cols(v, nch):
        # [nch*128] -> [128, nch] with chunk c in column c
        return np.ascontiguousarray(np.asarray(v, np.float32).reshape(nch, 128).T)

    common = {
        "w0": np.asarray(inputs["w0"], np.float32),          # [128, 512]
        "w1": np.asarray(inputs["w1"], np.float32),          # [512, 512]
        "w2": np.asarray(inputs["w2"], np.float32),
        "w3": np.asarray(inputs["w3"], np.float32),
        "fc1w": np.asarray(inputs["fc1_w"], np.float32),     # [512, 512]
        "fc2w": np.asarray(inputs["fc2_w"], np.float32),     # [512, 256]
        "fc3w": np.asarray(inputs["fc3_w"], np.float32),     # [256, 10]
        "b0c": chunk_cols(inputs["b0"], KC),
        "bb1c": chunk_cols(inputs["bb1"], KC),
        "bb2c": chunk_cols(inputs["bb2"], KC),
        "bb3c": chunk_cols(inputs["bb3"], KC),
        "fc1bc": chunk_cols(inputs["fc1_b"], KC),
        "fc2bc": chunk_cols(inputs["fc2_b"], 2),
        "fc3bc": np.asarray(inputs["fc3_b"], np.float32).reshape(C, 1),
        "bn1g": np.asarray(inputs["bn1_g"], np.float32).reshape(128, 1),
        "bn1b": np.asarray(inputs["bn1_b"], np.float32).reshape(128, 1),
        "bn3g": chunk_cols(inputs["bn3_g"], KC),
        "bn3b": chunk_cols(inputs["bn3_b"], KC),
        "a3v": np.asarray(inputs["a3"], np.float32).reshape(1, 1),
        "cnts": counts,
    }

    import ml_dtypes
    in_maps = []
    for c in range(NCORES):
        cp = pure_tiles[c * npure:(c + 1) * npure]
        cm = mixed_tiles[c * K:(c + 1) * K]
        arm = np.zeros((NTILES, G), np.float32)
        node_idx = []
        for t, (g, idx) in enumerate(cp):
            arm[t, g] = 1.0
            node_idx.append(idx)
        oh = np.zeros((125, K, KC, G), np.float32)
        for k, idx in enumerate(cm):
            node_idx.append(idx)
            gs = batch[idx]
            for pc in range(KC):
                seg = gs[pc * 125:(pc + 1) * 125]
                oh[np.arange(125), k, pc, seg] = 1.0
        perm = np.concatenate(node_idx)
        xt = np.ascontiguousarray(x[perm].T)  # [128, 12500]
        m = dict(common)
        m["xT"] = xt
        m["arm"] = arm
        m["oh"] = oh.astype(ml_dtypes.bfloat16)
        in_maps.append(m)
    return in_maps, K


def _declare_io(nc):
    specs = {
        "xT": ([IN, NS], F32),
        "w0": ([IN, D], F32),
        "w1": ([D, D], F32),
        "w2": ([D, D], F32),
        "w3": ([D, D], F32),
        "fc1w": ([D, D], F32),
        "fc2w": ([D, 256], F32),
        "fc3w": ([256, C], F32),
        "b0c": ([128, KC], F32),
        "bb1c": ([128, KC], F32),
        "bb2c": ([128, KC], F32),
        "bb3c": ([128, KC], F32),
        "fc1bc": ([128, KC], F32),
        "fc2bc": ([128, 2], F32),
        "fc3bc": ([C, 1], F32),
        "bn1g": ([128, 1], F32),
        "bn1b": ([128, 1], F32),
        "bn3g": ([128, KC], F32),
        "bn3b": ([128, KC], F32),
        "a3v": ([1, 1], F32),
        "cnts": ([1, G], F32),
        "arm": ([NTILES, G], F32),
        "oh": ([125, KMIX, KC, G], BF16),
    }
    ins = {k: nc.dram_tensor(k, shape, dt, kind="ExternalInput").ap()
           for k, (shape, dt) in specs.items()}
    out = nc.dram_tensor("out", [C, G], F32, kind="ExternalOutput").ap()
    return ins, out


def build_program(kmix):
    global KMIX
    KMIX = kmix
    nc = bacc.Bacc("TRN2", target_bir_lowering=False, debug=False,
                   enable_asserts=False, num_devices=NCORES)
    ins, out_ap = _declare_io(nc)
    with tile.TileContext(nc) as tc:
        _emit_kernel(nc, tc, ins, out_ap)
    nc.compile()
    return nc


def _emit_kernel(nc, tc, ins, out_ap):
    ctx = contextlib.ExitStack()
    with ctx:
        sbuf = ctx.enter_context(tc.tile_pool(name="sbuf", bufs=1))
        scratch = ctx.enter_context(tc.tile_pool(name="scratch", bufs=3))
        psum = ctx.enter_context(tc.tile_pool(name="psum", bufs=7, space="PSUM"))
        paux = ctx.enter_context(tc.tile_pool(name="paux", bufs=1, space="PSUM"))
        dram = ctx.enter_context(tc.tile_pool(name="dram", bufs=1, space="DRAM"))
        # weight staging ring: w1 -> w2 -> w3 -> fc1w reuse one 8KB slot, so
        # each load's DMA dispatches as soon as the previous tenant was cast
        # (mid-layer), never against a barrier.
        wstage = ctx.enter_context(tc.tile_pool(name="wstage", bufs=1))

        def stats_allreduce(sums_t, sq_t, nch):
            """Accum columns -> [128, nch, 2] -> AllReduce -> SBUF."""
            pack = sbuf.tile([128, nch, 2], F32, tag="statpack", name="pack")
            nc.vector.tensor_reduce(out=pack[:, :, 0], in_=sums_t, axis=AXX, op=ADD)
            nc.vector.tensor_reduce(out=pack[:, :, 1], in_=sq_t, axis=AXX, op=ADD)
            cin = dram.tile([128, nch, 2], F32, tag="ccin", name="cin")
            cout = dram.tile([128, nch, 2], F32, tag="ccout", name="cout")
            red = sbuf.tile([128, nch, 2], F32, tag="statred", name="red")
            nc.gpsimd.dma_start(out=cin, in_=pack)
            nc.gpsimd.collective_compute(
                "AllReduce", ADD, replica_groups=AR_GROUPS,
                ins=[cin.opt()], outs=[cout.opt()])
            nc.gpsimd.dma_start(out=red, in_=cout)
            return red

        def emit_s_t(red, nch, g_ap, b_ap):
            """s = g*rsqrt(var+eps), t = b - mean*s, both [128, nch] fp32."""
            m = sbuf.tile([128, nch], F32, tag="st_m", name="m")
            v = sbuf.tile([128, nch], F32, tag="st_v", name="v")
            s = sbuf.tile([128, nch], F32, tag="st_s", name="s")
            t = sbuf.tile([128, nch], F32, tag="st_t", name="t")
            nc.vector.tensor_scalar_mul(out=m, in0=red[:, :, 0], scalar1=1.0 / FN)
            nc.vector.tensor_scalar_mul(out=v, in0=red[:, :, 1], scalar1=1.0 / FN)
            nc.vector.tensor_tensor(out=s, in0=m, in1=m, op=MULT)
            nc.vector.tensor_sub(v, v, s)
            nc.scalar.activation(out=v, in_=v, func=Sqrt,
                                 bias=eps_t[:, 0:1], scale=1.0)
            nc.vector.reciprocal(out=s, in_=v)
            nc.vector.tensor_mul(s, s, g_ap)
            nc.vector.tensor_mul(m, m, s)
            nc.vector.tensor_sub(t, b_ap, m)
            return s, t

        def emit_w_cast(w_sb, s, name):
            """wf[:, kc, :] = w_sb[:, kc, :] * s[:, kc] -> bf16 [128, KC, D]."""
            wf = sbuf.tile([128, KC, D], BF16, tag="wf", name=name)
            for kc in range(KC):
                nc.vector.tensor_scalar_mul(
                    out=wf[:, kc, :], in0=w_sb[:, kc, :],
                    scalar1=s[:, kc:kc + 1])
            return wf

        def emit_bias_fold(wbf_chunks, t_ap, add_bias_ap, tag):
            """b' = t @ w + bias as [128, KC] via tiny bf16 PE matvecs."""
            nk = len(wbf_chunks)
            t_bf = sbuf.tile([128, nk], BF16, tag="tbf", name="t_bf")
            nc.vector.tensor_copy(out=t_bf, in_=t_ap)
            psb = paux.tile([128, KC], F32, tag="psb", name="psb")
            for dc in range(KC):
                for kc in range(nk):
                    nc.tensor.matmul(
                        psb[:, dc:dc + 1],
                        lhsT=wbf_chunks[kc][:, dc * 128:(dc + 1) * 128],
                        rhs=t_bf[:, kc:kc + 1],
                        start=(kc == 0), stop=(kc == nk - 1))
            bf = sbuf.tile([128, KC], F32, tag=tag, name=tag + "_bf")
            nc.vector.tensor_add(bf, psb, add_bias_ap)
            return bf

        def layer_matmuls(wf, j):
            """16 matmuls for node tile j: z[dc] = sum_kc wf[kc,dc].T @ R[kc,j]"""
            jsl = slice(j * NT, (j + 1) * NT)
            ps = []
            for dc in range(KC):
                p = psum.tile([128, NT], F32, tag="ps", name="p")
                for kc in range(KC):
                    nc.tensor.matmul(
                        p, lhsT=wf[:, kc, dc * 128:(dc + 1) * 128],
                        rhs=R[:, kc, jsl], start=(kc == 0), stop=(kc == KC - 1))
                ps.append(p)
            return ps

        def emit_mulreduce(in0, in1, accum, width, eng=None):
            """accum = sum(in0 * in1) along free axis (stt with dump)."""
            eng = eng or nc.vector
            dmp = scratch.tile([128, width], BF16, tag="qdump", name="dmp")
            eng.scalar_tensor_tensor(
                out=dmp[:, 0:width], in0=in0, scalar=1.0, in1=in1,
                op0=MULT, op1=MULT, accum_out=accum)

        def layer_drains(ps, bias_ap, Tacc, j, nact=3):
            """Drain 4 psum chunks: relu+bias -> R[:, dc, jsl] (in place);
            ACT takes the first `nact` chunks (with sum accum), DVE the rest
            (stt with accum)."""
            jsl = slice(j * NT, (j + 1) * NT)
            for dc in range(nact):
                acc = Tacc[:, dc, j:j + 1] if Tacc is not None else None
                nc.scalar.activation(
                    out=R[:, dc, jsl], in_=ps[dc], func=Relu,
                    bias=bias_ap[:, dc:dc + 1], scale=1.0, accum_out=acc)
            for dc in range(nact, KC):
                acc = Tacc[:, dc, j:j + 1] if Tacc is not None else None
                nc.vector.scalar_tensor_tensor(
                    out=R[:, dc, jsl], in0=ps[dc],
                    scalar=bias_ap[:, dc:dc + 1], in1=zeros_t,
                    op0=ADD, op1=MAX, accum_out=acc)

        def emit_q_spans(j, Qacc):
            """After tile j's drains: sumsq over the completed 2-tile span."""
            if j % 2 == 1 or j == NTILES - 1:
                sp = j // 2
                a = (j - 1) * NT if j % 2 == 1 else j * NT
                b = (j + 1) * NT
                for dc in range(KC):
                    emit_mulreduce(R[:, dc, a:b], R[:, dc, a:b],
                                   Qacc[:, dc, sp:sp + 1], b - a)

        NQSP = (NTILES + 1) // 2  # 13 sumsq spans per layer

        # ---------- resident hidden buffer (bf16, holds r0 -> r1 -> r2 -> r3)
        R = sbuf.tile([128, KC, NS], BF16, tag="R", name="R")

        # ---------- constants + activation-table preload ---------------------
        eps_t = sbuf.tile([128, 1], F32, tag="eps", name="eps_t")
        nc.vector.memset(eps_t, EPS)
        zeros_t = sbuf.tile([128, NT], F32, tag="zeros", name="zeros_t")
        nc.vector.memset(zeros_t, 0.0)
        for fn in (Relu, Copy, Sqrt, Sig):
            dmy = scratch.tile([128, 1], F32, tag="dummy", name="dmy")
            nc.scalar.activation(out=dmy, in_=eps_t, func=fn)

        qconst = nc.gpsimd if ALT_DMA_Q else nc.sync

        def load_const(key, shape, tag):
            t = sbuf.tile(shape, F32, tag=tag, name=tag)
            qconst.dma_start(out=t, in_=ins[key])
            return t

        # ================= P0: x streaming + bn1 stats =======================
        # x spans first on the Sync queue; consts on GpSimd; staged weights
        # follow x on Sync (their WAR deps resolve mid-layer, see wstage).
        with tc.tile_pool(name="w01pool", bufs=1) as w01pool, \
             tc.tile_pool(name="xstream", bufs=5) as xstream:
            x_bf = w01pool.tile([128, NS], BF16, tag="xbf", name="x_bf")
            T0 = sbuf.tile([128, NXSP], F32, tag="T0", name="T0")
            Q0 = sbuf.tile([128, NXSP], F32, tag="Q0", name="Q0")
            xts = []
            for sp in range(NXSP):
                a = sp * XSP
                xt = xstream.tile([128, XSP], F32, tag="xs", name="xt")
                nc.sync.dma_start(out=xt, in_=ins["xT"][:, a:a + XSP])
                xts.append((sp, a, xt))

            bn1g = load_const("bn1g", [128, 1], "bn1g")
            bn1b = load_const("bn1b", [128, 1], "bn1b")
            bn3g = load_const("bn3g", [128, KC], "bn3g")
            bn3b = load_const("bn3b", [128, KC], "bn3b")
            b0c = load_const("b0c", [128, KC], "b0c")
            bb1c = load_const("bb1c", [128, KC], "bb1c")
            bb2c = load_const("bb2c", [128, KC], "bb2c")
            bb3c = load_const("bb3c", [128, KC], "bb3c")

            w0_sb = w01pool.tile([128, D], F32, tag="w0", name="w0_sb")
            qconst.dma_start(out=w0_sb, in_=ins["w0"])
            wbf0 = w01pool.tile([128, D], BF16, tag="wbf0", name="wbf0")
            nc.vector.tensor_copy(out=wbf0, in_=w0_sb)

            w1_sb = wstage.tile([128, KC, D], F32, tag="wst", name="w1_sb")
            for kc in range(KC):
                nc.sync.dma_start(out=w1_sb[:, kc, :],
                                  in_=ins["w1"][kc * 128:(kc + 1) * 128, :])
            # plain bf16 cast of w1 (layer-1 input r0 has no preceding BN)
            w1f = w01pool.tile([128, KC, D], BF16, tag="w1f", name="w1f")
            for kc in range(KC):
                nc.vector.tensor_copy(out=w1f[:, kc, :], in_=w1_sb[:, kc, :])

            # stage w2 (slot WAR clears after the w1f cast, still in P0)
            w2_sb = wstage.tile([128, KC, D], F32, tag="wst", name="w2_sb")
            for kc in range(KC):
                nc.sync.dma_start(out=w2_sb[:, kc, :],
                                  in_=ins["w2"][kc * 128:(kc + 1) * 128, :])
            # small fc weights stream during P1 (fresh space, no WAR)
            fc2w_sb = wstage.tile([128, KC, 256], F32, tag="wst2",
                                  name="fc2w_sb")
            for kc in range(KC):
                nc.sync.dma_start(out=fc2w_sb[:, kc, :],
                                  in_=ins["fc2w"][kc * 128:(kc + 1) * 128, :])
            fc3w_sb = wstage.tile([128, 2, C], F32, tag="wst3", name="fc3w_sb")
            for kc in range(2):
                nc.sync.dma_start(out=fc3w_sb[:, kc, :],
                                  in_=ins["fc3w"][kc * 128:(kc + 1) * 128, :])

            # x stats: cast to bf16 on ACT (sum accum); sumsq on DVE, dumping
            # into R (overwritten by L0 anyway)
            for sp, a, xt in xts:
                nc.scalar.activation(out=x_bf[:, a:a + XSP], in_=xt,
                                     func=Copy, accum_out=T0[:, sp:sp + 1])
                nc.vector.scalar_tensor_tensor(
                    out=R[:, 1, a:a + XSP], in0=xt, scalar=1.0, in1=xt,
                    op0=MULT, op1=MULT, accum_out=Q0[:, sp:sp + 1])

            red1 = stats_allreduce(T0, Q0, 1)
            s1, t1 = emit_s_t(red1, 1, bn1g, bn1b)
            # fold bn1 into w0 (bf16, matching the bf16 copy of x)
            w0f = w01pool.tile([128, D], BF16, tag="w0f", name="w0f")
            nc.vector.tensor_scalar_mul(out=w0f, in0=w0_sb, scalar1=s1[:, 0:1])
            b0f = emit_bias_fold([wbf0], t1, b0c, "b0f")

            # ---- P1: fused L0+L1, software-pipelined by one tile ------------
            T1 = sbuf.tile([128, KC, NTILES], F32, tag="T1", name="T1")
            Q1 = sbuf.tile([128, KC, NQSP], F32, tag="Q1", name="Q1")

            def emit_l0(j):
                jsl = slice(j * NT, (j + 1) * NT)
                ps0 = []
                for dc in range(KC):
                    p = psum.tile([128, NT], F32, tag="ps", name="p")
                    nc.tensor.matmul(
                        p, lhsT=w0f[:, dc * 128:(dc + 1) * 128],
                        rhs=x_bf[:, jsl], start=True, stop=True)
                    ps0.append(p)
                for dc in range(2):
                    nc.scalar.activation(
                        out=R[:, dc, jsl], in_=ps0[dc], func=Relu,
                        bias=b0f[:, dc:dc + 1], scale=1.0)
                for dc in range(2, KC):
                    nc.vector.tensor_scalar(
                        out=R[:, dc, jsl], in0=ps0[dc],
                        scalar1=b0f[:, dc:dc + 1], scalar2=0.0,
                        op0=ADD, op1=MAX)

            def emit_l1(jj):
                ps = layer_matmuls(w1f, jj)
                layer_drains(ps, bb1c, T1, jj)
                emit_q_spans(jj, Q1)

            if FUSE_P1:
                for j in range(NTILES + 1):
                    if j < NTILES:
                        emit_l0(j)
                    if j >= 1:
                        emit_l1(j - 1)
            else:
                for j in range(NTILES):
                    emit_l0(j)
                for j in range(NTILES):
                    emit_l1(j)

        # ================= barrier #2 + L2 ===================================
        red2 = stats_allreduce(T1, Q1, KC)
        s2, t2 = emit_s_t(red2, KC, bn3g, bn3b)
        w2f = emit_w_cast(w2_sb, s2, "w2f")
        wbf2 = sbuf.tile([128, KC, D], BF16, tag="wbf", name="wbf2")
        for kc in range(KC):
            nc.vector.tensor_copy(out=wbf2[:, kc, :], in_=w2_sb[:, kc, :])
        b2f = emit_bias_fold([wbf2[:, kc, :] for kc in range(KC)],
                             t2, bb2c, "b2f")
        # stage w3 (slot WAR clears right here, at the w2 casts)
        w3_sb = wstage.tile([128, KC, D], F32, tag="wst", name="w3_sb")
        for kc in range(KC):
            nc.sync.dma_start(out=w3_sb[:, kc, :],
                              in_=ins["w3"][kc * 128:(kc + 1) * 128, :])
        T2 = sbuf.tile([128, KC, NTILES], F32, tag="T2", name="T2")
        Q2 = sbuf.tile([128, KC, NQSP], F32, tag="Q2", name="Q2")
        for j in range(NTILES):
            ps = layer_matmuls(w2f, j)
            layer_drains(ps, b2f, T2, j)
            emit_q_spans(j, Q2)

        # ================= barrier #3 + L3 + pooling + FC head ===============
        with tc.tile_pool(name="fcpool", bufs=1) as fcpool:
            arm_sb = fcpool.tile([NTILES, G], F32, tag="arm", name="arm_sb")
            qconst.dma_start(out=arm_sb, in_=ins["arm"])
            oh_sb = fcpool.tile([125, KMIX, KC, G], BF16, tag="oh", name="oh_sb")
            qconst.dma_start(out=oh_sb, in_=ins["oh"])
            ident = fcpool.tile([128, 128], F32, tag="ident", name="ident")
            from concourse.masks import make_identity
            make_identity(nc, ident)
            identG = fcpool.tile([G, G], F32, tag="identG", name="identG")
            make_identity(nc, identG)
            identb = fcpool.tile([128, 128], BF16, tag="identb", name="identb")
            nc.vector.tensor_copy(out=identb, in_=ident)
            ngb = fcpool.tile([128, G], F32, tag="ngb", name="ngb")
            qconst.dma_start(out=ngb, in_=ins["cnts"].to_broadcast([128, G]))
            a3b = fcpool.tile([128, 1], F32, tag="a3b", name="a3b")
            qconst.dma_start(out=a3b, in_=ins["a3v"].to_broadcast([128, 1]))
            fc1bc_sb = fcpool.tile([128, KC], F32, tag="fc1bc", name="fc1bc_sb")
            qconst.dma_start(out=fc1bc_sb, in_=ins["fc1bc"])
            fc2bc_sb = fcpool.tile([128, 2], F32, tag="fc2bc", name="fc2bc_sb")
            qconst.dma_start(out=fc2bc_sb, in_=ins["fc2bc"])
            fc3bc_sb = fcpool.tile([C, 1], F32, tag="fc3bc", name="fc3bc_sb")
            qconst.dma_start(out=fc3bc_sb, in_=ins["fc3bc"])

            red3 = stats_allreduce(T2, Q2, KC)
            s3, t3 = emit_s_t(red3, KC, bn3g, bn3b)
            w3f = emit_w_cast(w3_sb, s3, "w3f")
            wbf3 = sbuf.tile([128, KC, D], BF16, tag="wbf", name="wbf3")
            for kc in range(KC):
                nc.vector.tensor_copy(out=wbf3[:, kc, :], in_=w3_sb[:, kc, :])
            b3f = emit_bias_fold([wbf3[:, kc, :] for kc in range(KC)],
                                 t3, bb3c, "b3f")
            # stage fc1w (slot WAR clears at the w3 casts); bf16 cast follows
            fc1w_sb = wstage.tile([128, KC, D], F32, tag="wst", name="fc1w_sb")
            for kc in range(KC):
                nc.sync.dma_start(out=fc1w_sb[:, kc, :],
                                  in_=ins["fc1w"][kc * 128:(kc + 1) * 128, :])

            # ---- L3: r3 -> R in place; per-tile sums + sumsq ----------------
            TC = sbuf.tile([128, KC, NTILES], F32, tag="TC", name="TC")
            Q3 = sbuf.tile([128, KC, NQSP], F32, tag="Q3", name="Q3")
            for j in range(NTILES):
                ps = layer_matmuls(w3f, j)
                layer_drains(ps, b3f, TC, j)
                emit_q_spans(j, Q3)

            # bf16 copies of the fc weights (DMA'd during L3)
            fc1wb = fcpool.tile([128, KC, D], BF16, tag="fc1wb", name="fc1wb")
            for kc in range(KC):
                nc.vector.tensor_copy(out=fc1wb[:, kc, :], in_=fc1w_sb[:, kc, :])
            fc2wb = fcpool.tile([128, KC, 256], BF16, tag="fc2wb", name="fc2wb")
            for kc in range(KC):
                nc.vector.tensor_copy(out=fc2wb[:, kc, :], in_=fc2w_sb[:, kc, :])
            fc3wb = fcpool.tile([128, 2, C], BF16, tag="fc3wb", name="fc3wb")
            for kc in range(2):
                nc.vector.tensor_copy(out=fc3wb[:, kc, :], in_=fc3w_sb[:, kc, :])

            # pooled partials: arm.T @ TC.T (pure tiles, fp32) plus the
            # node-level one-hot matmul for the K mixed tiles (bf16 r3).
            poolG = fcpool.tile([G, KC, 128], F32, tag="poolG", name="poolG")
            mix0 = (NTILES - KMIX) * NT
            for dc in range(KC):
                pT = psum.tile([NTILES, 128], F32, tag="ps", name="pT")
                nc.tensor.transpose(pT, TC[:, dc, :], ident)
                tct = scratch.tile([NTILES, 128], F32, tag="tct",
                                   name="tct")
                nc.vector.tensor_copy(out=tct, in_=pT)
                pG = psum.tile([G, 128], F32, tag="ps", name="pG")
                nc.tensor.matmul(pG, lhsT=arm_sb, rhs=tct, start=True, stop=True)
                # mixed tiles: transpose r3 [128,125] pieces, one-hot matmul
                pM = psum.tile([G, 128], F32, tag="ps", name="pM")
                nmm = KMIX * KC
                i = 0
                for k in range(KMIX):
                    for pc in range(KC):
                        a = mix0 + k * NT + pc * 125
                        pR = psum.tile([125, 128], BF16, tag="ps", name="pR")
                        nc.tensor.transpose(pR, R[:, dc, a:a + 125], identb)
                        r3t = scratch.tile([125, 128], BF16, tag="r3t",
                                           name="r3t")
                        nc.vector.tensor_copy(out=r3t, in_=pR)
                        nc.tensor.matmul(pM, lhsT=oh_sb[:, k, pc, :], rhs=r3t,
                                         start=(i == 0), stop=(i == nmm - 1))
                        i += 1
                pMs = scratch.tile([G, 128], F32, tag="pms", name="pMs")
                nc.vector.tensor_copy(out=pMs, in_=pM)
                nc.vector.tensor_tensor(out=poolG[:, dc, :], in0=pG, in1=pMs,
                                        op=ADD)

            # pack bn3#3 stats + pooled partials into one AllReduce
            nst = 128 * KC * 2
            flat = dram.tile([nst + G * KC * 128], F32, tag="cc4in", name="flat")
            flat_out = dram.tile([nst + G * KC * 128], F32, tag="cc4out",
                                 name="flat_out")
            pack = sbuf.tile([128, KC, 2], F32, tag="statpack", name="pack4")
            nc.vector.tensor_reduce(out=pack[:, :, 0], in_=TC,
                                    axis=AXX, op=ADD)
            nc.vector.tensor_reduce(out=pack[:, :, 1], in_=Q3, axis=AXX, op=ADD)
            red4 = sbuf.tile([128, KC, 2], F32, tag="statred", name="red4")
            poolGr = fcpool.tile([G, KC, 128], F32, tag="poolGr", name="poolGr")
            nc.gpsimd.dma_start(
                out=flat[0:nst].rearrange("(p c k) -> p c k", p=128, c=KC),
                in_=pack)
            nc.gpsimd.dma_start(
                out=flat[nst:].rearrange("(g c f) -> g c f", g=G, c=KC),
                in_=poolG)
            nc.gpsimd.collective_compute(
                "AllReduce", ADD, replica_groups=AR_GROUPS,
                ins=[flat.opt()], outs=[flat_out.opt()])
            nc.gpsimd.dma_start(
                out=red4,
                in_=flat_out[0:nst].rearrange("(p c k) -> p c k", p=128, c=KC))
            nc.gpsimd.dma_start(
                out=poolGr,
                in_=flat_out[nst:].rearrange("(g c f) -> g c f", g=G, c=KC))

            s4, t4 = emit_s_t(red4, KC, bn3g, bn3b)

            # pooled_bn[f, g] = s4[f]*pooled[f, g] + t4[f]*n[g] (feature-major,
            # bf16 for the fc matmuls)
            pooled = fcpool.tile([128, KC, G], BF16, tag="pooled", name="pooled")
            for dc in range(KC):
                pF = psum.tile([128, G], F32, tag="ps", name="pF")
                nc.tensor.transpose(pF, poolGr[:, dc, :], identG)
                tmp = scratch.tile([128, G], F32, tag="ngt", name="tmp")
                nc.vector.tensor_scalar(
                    out=tmp, in0=pF, scalar1=s4[:, dc:dc + 1],
                    scalar2=None, op0=MULT)
                tmp2 = scratch.tile([128, G], F32, tag="ngt2", name="tmp2")
                nc.vector.tensor_scalar(
                    out=tmp2, in0=ngb, scalar1=t4[:, dc:dc + 1],
                    scalar2=None, op0=MULT)
                nc.vector.tensor_add(pooled[:, dc, :], tmp, tmp2)

            # ---------------- FC head (bf16 matmuls) -------------------------
            h1 = fcpool.tile([128, KC, G], BF16, tag="h1", name="h1")
            for dc in range(KC):
                p = psum.tile([128, G], F32, tag="ps", name="pfc")
                for kc in range(KC):
                    nc.tensor.matmul(
                        p, lhsT=fc1wb[:, kc, dc * 128:(dc + 1) * 128],
                        rhs=pooled[:, kc, :], start=(kc == 0), stop=(kc == KC - 1))
                z = scratch.tile([128, G], F32, tag="fcz", name="z")
                nc.vector.tensor_scalar(out=z, in0=p,
                                        scalar1=fc1bc_sb[:, dc:dc + 1],
                                        scalar2=None, op0=ADD)
                # prelu(z) = max(a*z, z) for 0 <= a <= 1
                nc.vector.scalar_tensor_tensor(
                    out=h1[:, dc, :], in0=z, scalar=a3b[:, 0:1], in1=z,
                    op0=MULT, op1=MAX)

            # fc2 + sigmoid -> h2 [128, 2, G]
            h2 = fcpool.tile([128, 2, G], BF16, tag="h2", name="h2")
            for ec in range(2):
                p = psum.tile([128, G], F32, tag="ps", name="pfc2")
                for kc in range(KC):
                    nc.tensor.matmul(
                        p, lhsT=fc2wb[:, kc, ec * 128:(ec + 1) * 128],
                        rhs=h1[:, kc, :], start=(kc == 0), stop=(kc == KC - 1))
                nc.scalar.activation(out=h2[:, ec, :], in_=p, func=Sig,
                                     bias=fc2bc_sb[:, ec:ec + 1], scale=1.0)

            # fc3 -> out [10, 64]
            p = psum.tile([C, G], F32, tag="ps", name="pfc3")
            for kc in range(2):
                nc.tensor.matmul(p, lhsT=fc3wb[:, kc, :], rhs=h2[:, kc, :],
                                 start=(kc == 0), stop=(kc == 1))
            ob = fcpool.tile([C, G], F32, tag="ob", name="ob")
            nc.vector.tensor_scalar(out=ob, in0=p, scalar1=fc3bc_sb,
                                    scalar2=None, op0=ADD)
            nc.sync.dma_start(out=out_ap, in_=ob)


_cached = {}


def kernel(**inputs) -> np.ndarray:
    in_maps, kmix = _build_host_inputs(inputs)
    if _cached.get("kmix") != kmix:
        _cached["nc"] = build_program(kmix)
        _cached["kmix"] = kmix
    nc = _cached["nc"]
    res = bass_utils.run_bass_kernel_spmd(
        nc, in_maps, core_ids=list(range(NCORES)))
    out = res.results[0]["out"]  # [10, 64]
    return np.ascontiguousarray(out.T.astype(np.float32))


if __name__ == "__main__":
    import reference
    inp = {k: np.asarray(v) for k, v in reference.setup_inputs().items()}
    got = kernel(**inp)
    exp = np.asarray(reference.reference(**{
        k: np.asarray(v) for k, v in reference.setup_inputs().items()}))
    err = np.linalg.norm(got - exp) / np.linalg.norm(exp)
    print("Relative error:", err)



# revision 10
# speedup vs baseline: 1.1041x; 1.1041x over previous
"""Trainium2 Bass kernel for a 4-layer GNN-style MLP (ChebConv K=1) with
training-mode BatchNorm, global_add_pool over 64 graphs, and a 3-layer FC head.

Strategy (8 NeuronCores, data-parallel over nodes):
  - 12500 nodes/core, feature-major layout [feat_part(128) x nodes_free] so the
    whole matmul chain needs no transposes.
  - BatchNorm (batch statistics over all 100k nodes) is folded into the next
    matmul's weights: bn(h) @ w + b == h @ (s*w) + (t@w + b).  Per-feature
    sum/sumsq are accumulated on-chip and combined across cores with small
    AllReduces.
  - Engine balance per node-tile keeps the PE free of drain stalls (which
    would hold the PE at its low p-state clock): ACT drains 3 of 4 relu
    chunks (with sum accumulators), DVE drains 1 chunk and computes sumsq
    via tensor_tensor_reduce on the bf16 activations.
  - L0 (128->512) and L1 (512->512) are fused into one software-pipelined
    tile loop (L0 of tile j runs while L1 of tile j-1 drains).
  - Pooling: per-tile node sums come free from the relu accumulators;
    graph-boundary suffixes are corrected with per-tile 0/1 masks
    (tensor_tensor_reduce), then a tiny one-hot matmul scatters tile sums
    into the 64 graph bins; bn3's affine is applied post-AllReduce.
"""

import contextlib

import numpy as np

import concourse.bass as bass
import concourse.tile as tile
from concourse import bacc, mybir
from concourse import bass_utils

F32 = mybir.dt.float32
BF16 = mybir.dt.bfloat16

# Problem constants (hardcoded per contract).
N = 100000          # nodes
IN = 128            # input features
D = 512             # hidden dim
G = 64              # graphs
C = 10              # classes
EPS = 1e-5
NCORES = 8
NS = N // NCORES    # nodes per core = 12500
NT = 500            # node tile (free dim per matmul)
NTILES = NS // NT   # 25
KC = D // 128       # 4 chunks of the hidden dim
FN = float(N)
XSP = 1250          # x streaming span
NXSP = NS // XSP    # 5

AR_GROUPS = [list(range(NCORES))]
KMIX = 1  # mixed (multi-graph) tiles per core; set by build_program

import os
_V = set(os.environ.get("KERNEL_V", "").split(",")) - {""}
# tensor_tensor_reduce hangs TRN2 hardware in this codegen path (bisected
# 2026-08-08); default to the scalar_tensor_tensor fallback.
USE_TTR = "ttr" in _V
FUSE_P1 = "nofuse" not in _V     # pipeline L0+L1 in one tile loop
ALT_DMA_Q = "syncdma" not in _V  # consts on gpsimd DMA queue
POOL_STT = False  # walrus rejects TensorScalarPtr on the Pool engine

Relu = mybir.ActivationFunctionType.Relu
Copy = mybir.ActivationFunctionType.Copy
Sqrt = mybir.ActivationFunctionType.Sqrt
Sig = mybir.ActivationFunctionType.Sigmoid
ADD = mybir.AluOpType.add
MULT = mybir.AluOpType.mult
MAX = mybir.AluOpType.max
AXX = mybir.AxisListType.X


def _bcast_part(ap, nparts):
    """Stride-0 partition broadcast of a DRAM AP: [a, b] -> [nparts, a, b]."""
    return bass.AP(tensor=ap.tensor, offset=ap.offset,
                   ap=[[0, nparts]] + list(ap.ap))


def _build_host_inputs(inputs):
    """Shard + reshape the full problem inputs into per-core input maps.

    Nodes are permuted so that all but K tiles per core hold nodes of a
    single graph (pure tiles -> one-hot row in `arm`); the graph-remainder
    nodes are packed into K "mixed" tiles per core, pooled on-device via a
    node-level one-hot matmul (`oh`).  This removes the suffix-mask pass.
    """
    x = np.asarray(inputs["x"], np.float32)
    batch = np.asarray(inputs["batch"]).astype(np.int64)

    # bn1 is a pure function of the raw input -> fold it into x host-side
    # (the AllReduce + stats pass for bn1 dominated device-side startup).
    xm = x.mean(0, dtype=np.float64)
    xv = x.var(0, dtype=np.float64)
    s1 = (np.asarray(inputs["bn1_g"], np.float64) / np.sqrt(xv + EPS))
    t1 = np.asarray(inputs["bn1_b"], np.float64) - xm * s1
    x = (x * s1 + t1).astype(np.float32)

    counts = np.bincount(batch, minlength=G).astype(np.float32).reshape(1, G)

    # node indices per graph (batch is sorted)
    starts = np.searchsorted(batch, np.arange(G + 1))
    pure_tiles = []   # (graph, node_index_array)
    rem_idx = []
    for g in range(G):
        idx = np.arange(starts[g], starts[g + 1])
        nfull = len(idx) // NT
        for t in range(nfull):
            pure_tiles.append((g, idx[t * NT:(t + 1) * NT]))
        rem_idx.append(idx[nfull * NT:])
    rem = np.concatenate(rem_idx)
    assert len(rem) % NT == 0
    mixed_tiles = [rem[i * NT:(i + 1) * NT] for i in range(len(rem) // NT)]
    M = len(mixed_tiles)
    K = (M + NCORES - 1) // NCORES
    # reclassify pure tiles as mixed to give every core exactly K mixed
    while len(mixed_tiles) < K * NCORES:
        g, idx = pure_tiles.pop()
        mixed_tiles.append(idx)
    npure = NTILES - K

    def chunk_cols(v, nch):
        # [nch*128] -> [128, nch] with chunk c in column c
        return np.ascontiguousarray(np.asarray(v, np.float32).reshape(nch, 128).T)

    common = {
        "w0": np.asarray(inputs["w0"], np.float32),          # [128, 512]
        "w1": np.asarray(inputs["w1"], np.float32),          # [512, 512]
        "w2": np.asarray(inputs["w2"], np.float32),
        "w3": np.asarray(inputs["w3"], np.float32),
        "fc1w": np.asarray(inputs["fc1_w"], np.float32),     # [512, 512]
        "fc2w": np.asarray(inputs["fc2_w"], np.float32),     # [512, 256]
        "fc3w": np.asarray(inputs["fc3_w"], np.float32),     # [256, 10]
        "b0c": chunk_cols(inputs["b0"], KC),
        "bb1c": chunk_cols(inputs["bb1"], KC),
        "bb2c": chunk_cols(inputs["bb2"], KC),
        "bb3c": chunk_cols(inputs["bb3"], KC),
        "fc1bc": chunk_cols(inputs["fc1_b"], KC),
        "fc2bc": chunk_cols(inputs["fc2_b"], 2),
        "fc3bc": np.asarray(inputs["fc3_b"], np.float32).reshape(C, 1),
        "bn3g": chunk_cols(inputs["bn3_g"], KC),
        "bn3b": chunk_cols(inputs["bn3_b"], KC),
        "a3v": np.asarray(inputs["a3"], np.float32).reshape(1, 1),
        "cnts": counts,
    }

    import ml_dtypes
    in_maps = []
    for c in range(NCORES):
        cp = pure_tiles[c * npure:(c + 1) * npure]
        cm = mixed_tiles[c * K:(c + 1) * K]
        arm = np.zeros((NTILES, G), np.float32)
        node_idx = []
        for t, (g, idx) in enumerate(cp):
            arm[t, g] = 1.0
            node_idx.append(idx)
        oh = np.zeros((125, K, KC, G), np.float32)
        for k, idx in enumerate(cm):
            node_idx.append(idx)
            gs = batch[idx]
            for pc in range(KC):
                seg = gs[pc * 125:(pc + 1) * 125]
                oh[np.arange(125), k, pc, seg] = 1.0
        perm = np.concatenate(node_idx)
        xt = np.ascontiguousarray(x[perm].T).astype(ml_dtypes.bfloat16)
        m = dict(common)
        m["xT"] = xt
        m["arm"] = arm
        m["oh"] = oh.astype(ml_dtypes.bfloat16)
        in_maps.append(m)
    return in_maps, K


def _declare_io(nc):
    specs = {
        "xT": ([IN, NS], BF16),
        "w0": ([IN, D], F32),
        "w1": ([D, D], F32),
        "w2": ([D, D], F32),
        "w3": ([D, D], F32),
        "fc1w": ([D, D], F32),
        "fc2w": ([D, 256], F32),
        "fc3w": ([256, C], F32),
        "b0c": ([128, KC], F32),
        "bb1c": ([128, KC], F32),
        "bb2c": ([128, KC], F32),
        "bb3c": ([128, KC], F32),
        "fc1bc": ([128, KC], F32),
        "fc2bc": ([128, 2], F32),
        "fc3bc": ([C, 1], F32),
        "bn3g": ([128, KC], F32),
        "bn3b": ([128, KC], F32),
        "a3v": ([1, 1], F32),
        "cnts": ([1, G], F32),
        "arm": ([NTILES, G], F32),
        "oh": ([125, KMIX, KC, G], BF16),
    }
    ins = {k: nc.dram_tensor(k, shape, dt, kind="ExternalInput").ap()
           for k, (shape, dt) in specs.items()}
    out = nc.dram_tensor("out", [C, G], F32, kind="ExternalOutput").ap()
    return ins, out


def build_program(kmix):
    global KMIX
    KMIX = kmix
    nc = bacc.Bacc("TRN2", target_bir_lowering=False, debug=False,
                   enable_asserts=False, num_devices=NCORES)
    ins, out_ap = _declare_io(nc)
    with tile.TileContext(nc) as tc:
        _emit_kernel(nc, tc, ins, out_ap)
    nc.compile()
    return nc


def _emit_kernel(nc, tc, ins, out_ap):
    ctx = contextlib.ExitStack()
    with ctx:
        sbuf = ctx.enter_context(tc.tile_pool(name="sbuf", bufs=1))
        scratch = ctx.enter_context(tc.tile_pool(name="scratch", bufs=3))
        psum = ctx.enter_context(tc.tile_pool(name="psum", bufs=7, space="PSUM"))
        paux = ctx.enter_context(tc.tile_pool(name="paux", bufs=1, space="PSUM"))
        dram = ctx.enter_context(tc.tile_pool(name="dram", bufs=1, space="DRAM"))
        # weight staging ring: w1 -> w2 -> w3 -> fc1w reuse one 8KB slot, so
        # each load's DMA dispatches as soon as the previous tenant was cast
        # (mid-layer), never against a barrier.
        wstage = ctx.enter_context(tc.tile_pool(name="wstage", bufs=1))

        def stats_allreduce(sums_t, sq_t, nch):
            """Accum columns -> [128, nch, 2] -> AllReduce -> SBUF."""
            pack = sbuf.tile([128, nch, 2], F32, tag="statpack", name="pack")
            nc.vector.tensor_reduce(out=pack[:, :, 0], in_=sums_t, axis=AXX, op=ADD)
            nc.vector.tensor_reduce(out=pack[:, :, 1], in_=sq_t, axis=AXX, op=ADD)
            cin = dram.tile([128, nch, 2], F32, tag="ccin", name="cin")
            cout = dram.tile([128, nch, 2], F32, tag="ccout", name="cout")
            red = sbuf.tile([128, nch, 2], F32, tag="statred", name="red")
            nc.gpsimd.dma_start(out=cin, in_=pack)
            nc.gpsimd.collective_compute(
                "AllReduce", ADD, replica_groups=AR_GROUPS,
                ins=[cin.opt()], outs=[cout.opt()])
            nc.gpsimd.dma_start(out=red, in_=cout)
            return red

        def emit_s_t(red, nch, g_ap, b_ap):
            """s = g*rsqrt(var+eps), t = b - mean*s, both [128, nch] fp32."""
            m = sbuf.tile([128, nch], F32, tag="st_m", name="m")
            v = sbuf.tile([128, nch], F32, tag="st_v", name="v")
            s = sbuf.tile([128, nch], F32, tag="st_s", name="s")
            t = sbuf.tile([128, nch], F32, tag="st_t", name="t")
            nc.vector.tensor_scalar_mul(out=m, in0=red[:, :, 0], scalar1=1.0 / FN)
            nc.vector.tensor_scalar_mul(out=v, in0=red[:, :, 1], scalar1=1.0 / FN)
            nc.vector.tensor_tensor(out=s, in0=m, in1=m, op=MULT)
            nc.vector.tensor_sub(v, v, s)
            nc.scalar.activation(out=v, in_=v, func=Sqrt,
                                 bias=eps_t[:, 0:1], scale=1.0)
            nc.vector.reciprocal(out=s, in_=v)
            nc.vector.tensor_mul(s, s, g_ap)
            nc.vector.tensor_mul(m, m, s)
            nc.vector.tensor_sub(t, b_ap, m)
            return s, t

        def emit_w_cast(w_sb, s, name):
            """wf[:, kc, :] = w_sb[:, kc, :] * s[:, kc] -> bf16 [128, KC, D]."""
            wf = sbuf.tile([128, KC, D], BF16, tag="wf", name=name)
            for kc in range(KC):
                nc.vector.tensor_scalar_mul(
                    out=wf[:, kc, :], in0=w_sb[:, kc, :],
                    scalar1=s[:, kc:kc + 1])
            return wf

        def emit_bias_fold(wbf_chunks, t_ap, add_bias_ap, tag):
            """b' = t @ w + bias as [128, KC] via tiny bf16 PE matvecs."""
            nk = len(wbf_chunks)
            t_bf = sbuf.tile([128, nk], BF16, tag="tbf", name="t_bf")
            nc.vector.tensor_copy(out=t_bf, in_=t_ap)
            psb = paux.tile([128, KC], F32, tag="psb", name="psb")
            for dc in range(KC):
                for kc in range(nk):
                    nc.tensor.matmul(
                        psb[:, dc:dc + 1],
                        lhsT=wbf_chunks[kc][:, dc * 128:(dc + 1) * 128],
                        rhs=t_bf[:, kc:kc + 1],
                        start=(kc == 0), stop=(kc == nk - 1))
            bf = sbuf.tile([128, KC], F32, tag=tag, name=tag + "_bf")
            nc.vector.tensor_add(bf, psb, add_bias_ap)
            return bf

        def layer_matmuls(wf, j):
            """16 matmuls for node tile j: z[dc] = sum_kc wf[kc,dc].T @ R[kc,j]"""
            jsl = slice(j * NT, (j + 1) * NT)
            ps = []
            for dc in range(KC):
                p = psum.tile([128, NT], F32, tag="ps", name="p")
                for kc in range(KC):
                    nc.tensor.matmul(
                        p, lhsT=wf[:, kc, dc * 128:(dc + 1) * 128],
                        rhs=R[:, kc, jsl], start=(kc == 0), stop=(kc == KC - 1))
                ps.append(p)
            return ps

        def emit_mulreduce(in0, in1, accum, width, eng=None):
            """accum = sum(in0 * in1) along free axis (stt with dump)."""
            eng = eng or nc.vector
            dmp = scratch.tile([128, width], BF16, tag="qdump", name="dmp")
            eng.scalar_tensor_tensor(
                out=dmp[:, 0:width], in0=in0, scalar=1.0, in1=in1,
                op0=MULT, op1=MULT, accum_out=accum)

        def layer_drains(ps, bias_ap, Tacc, j, nact=3):
            """Drain 4 psum chunks: relu+bias -> R[:, dc, jsl] (in place);
            ACT takes the first `nact` chunks (with sum accum), DVE the rest
            (stt with accum)."""
            jsl = slice(j * NT, (j + 1) * NT)
            for dc in range(nact):
                acc = Tacc[:, dc, j:j + 1] if Tacc is not None else None
                nc.scalar.activation(
                    out=R[:, dc, jsl], in_=ps[dc], func=Relu,
                    bias=bias_ap[:, dc:dc + 1], scale=1.0, accum_out=acc)
            for dc in range(nact, KC):
                acc = Tacc[:, dc, j:j + 1] if Tacc is not None else None
                nc.vector.scalar_tensor_tensor(
                    out=R[:, dc, jsl], in0=ps[dc],
                    scalar=bias_ap[:, dc:dc + 1], in1=zeros_t,
                    op0=ADD, op1=MAX, accum_out=acc)

        def emit_q_spans(j, Qacc):
            """After tile j's drains: sumsq over the completed 2-tile span."""
            if j % 2 == 1 or j == NTILES - 1:
                sp = j // 2
                a = (j - 1) * NT if j % 2 == 1 else j * NT
                b = (j + 1) * NT
                for dc in range(KC):
                    emit_mulreduce(R[:, dc, a:b], R[:, dc, a:b],
                                   Qacc[:, dc, sp:sp + 1], b - a)

        NQSP = (NTILES + 1) // 2  # 13 sumsq spans per layer

        # ---------- resident hidden buffer (bf16, holds r0 -> r1 -> r2 -> r3)
        R = sbuf.tile([128, KC, NS], BF16, tag="R", name="R")

        # ---------- constants + activation-table preload ---------------------
        eps_t = sbuf.tile([128, 1], F32, tag="eps", name="eps_t")
        nc.vector.memset(eps_t, EPS)
        zeros_t = sbuf.tile([128, NT], F32, tag="zeros", name="zeros_t")
        nc.vector.memset(zeros_t, 0.0)
        for fn in (Relu, Copy, Sqrt, Sig):
            dmy = scratch.tile([128, 1], F32, tag="dummy", name="dmy")
            nc.scalar.activation(out=dmy, in_=eps_t, func=fn)

        qconst = nc.gpsimd if ALT_DMA_Q else nc.sync

        def load_const(key, shape, tag):
            t = sbuf.tile(shape, F32, tag=tag, name=tag)
            qconst.dma_start(out=t, in_=ins[key])
            return t

        # ================= P0: x streaming (bn1 folded on host) ==============
        # x spans stream straight into the resident bf16 buffer on the Sync
        # queue; consts on GpSimd; staged weights follow x on Sync.
        with tc.tile_pool(name="w01pool", bufs=1) as w01pool:
            x_bf = w01pool.tile([128, NS], BF16, tag="xbf", name="x_bf")
            for sp in range(NXSP):
                a = sp * XSP
                nc.sync.dma_start(out=x_bf[:, a:a + XSP],
                                  in_=ins["xT"][:, a:a + XSP])

            bn3g = load_const("bn3g", [128, KC], "bn3g")
            bn3b = load_const("bn3b", [128, KC], "bn3b")
            b0c = load_const("b0c", [128, KC], "b0c")
            bb1c = load_const("bb1c", [128, KC], "bb1c")
            bb2c = load_const("bb2c", [128, KC], "bb2c")
            bb3c = load_const("bb3c", [128, KC], "bb3c")

            w0_sb = w01pool.tile([128, D], F32, tag="w0", name="w0_sb")
            qconst.dma_start(out=w0_sb, in_=ins["w0"])
            w0f = w01pool.tile([128, D], BF16, tag="wbf0", name="w0f")
            nc.vector.tensor_copy(out=w0f, in_=w0_sb)

            # absorb first-collective entry sync (cross-core launch skew)
            # with a throwaway AllReduce fired at t~0 on the GpSimd queue.
            dmy_in = dram.tile([8], F32, tag="dmyi", name="dmy_in")
            dmy_out = dram.tile([8], F32, tag="dmyo", name="dmy_out")
            dmy_sb = scratch.tile([1, 8], F32, tag="dmysb", name="dmy_sb")
            nc.gpsimd.memset(dmy_sb, 0.0)
            nc.gpsimd.dma_start(out=dmy_in.rearrange("(a b) -> a b", a=1),
                                in_=dmy_sb)
            nc.gpsimd.collective_compute(
                "AllReduce", ADD, replica_groups=AR_GROUPS,
                ins=[dmy_in.opt()], outs=[dmy_out.opt()])

            w1_sb = wstage.tile([128, KC, D], F32, tag="wst", name="w1_sb")
            for kc in range(KC):
                nc.sync.dma_start(out=w1_sb[:, kc, :],
                                  in_=ins["w1"][kc * 128:(kc + 1) * 128, :])
            # plain bf16 cast of w1 (layer-1 input r0 has no preceding BN)
            w1f = w01pool.tile([128, KC, D], BF16, tag="w1f", name="w1f")
            for kc in range(KC):
                nc.vector.tensor_copy(out=w1f[:, kc, :], in_=w1_sb[:, kc, :])

            # stage w2 (slot WAR clears after the w1f cast, still in P0)
            w2_sb = wstage.tile([128, KC, D], F32, tag="wst", name="w2_sb")
            for kc in range(KC):
                nc.sync.dma_start(out=w2_sb[:, kc, :],
                                  in_=ins["w2"][kc * 128:(kc + 1) * 128, :])
            # small fc weights stream during P1 (fresh space, no WAR)
            fc2w_sb = wstage.tile([128, KC, 256], F32, tag="wst2",
                                  name="fc2w_sb")
            for kc in range(KC):
                nc.sync.dma_start(out=fc2w_sb[:, kc, :],
                                  in_=ins["fc2w"][kc * 128:(kc + 1) * 128, :])
            fc3w_sb = wstage.tile([128, 2, C], F32, tag="wst3", name="fc3w_sb")
            for kc in range(2):
                nc.sync.dma_start(out=fc3w_sb[:, kc, :],
                                  in_=ins["fc3w"][kc * 128:(kc + 1) * 128, :])

            # ---- P1: fused L0+L1, software-pipelined by one tile ------------
            T1 = sbuf.tile([128, KC, NTILES], F32, tag="T1", name="T1")
            Q1 = sbuf.tile([128, KC, NQSP], F32, tag="Q1", name="Q1")

            def emit_l0(j):
                jsl = slice(j * NT, (j + 1) * NT)
                ps0 = []
                for dc in range(KC):
                    p = psum.tile([128, NT], F32, tag="ps", name="p")
                    nc.tensor.matmul(
                        p, lhsT=w0f[:, dc * 128:(dc + 1) * 128],
                        rhs=x_bf[:, jsl], start=True, stop=True)
                    ps0.append(p)
                for dc in range(2):
                    nc.scalar.activation(
                        out=R[:, dc, jsl], in_=ps0[dc], func=Relu,
                        bias=b0c[:, dc:dc + 1], scale=1.0)
                for dc in range(2, KC):
                    nc.vector.tensor_scalar(
                        out=R[:, dc, jsl], in0=ps0[dc],
                        scalar1=b0c[:, dc:dc + 1], scalar2=0.0,
                        op0=ADD, op1=MAX)

            def emit_l1(jj):
                ps = layer_matmuls(w1f, jj)
                layer_drains(ps, bb1c, T1, jj)
                emit_q_spans(jj, Q1)

            if FUSE_P1:
                for j in range(NTILES + 1):
                    if j < NTILES:
                        emit_l0(j)
                    if j >= 1:
                        emit_l1(j - 1)
            else:
                for j in range(NTILES):
                    emit_l0(j)
                for j in range(NTILES):
                    emit_l1(j)

        # ================= barrier #2 + L2 ===================================
        red2 = stats_allreduce(T1, Q1, KC)
        s2, t2 = emit_s_t(red2, KC, bn3g, bn3b)
        w2f = emit_w_cast(w2_sb, s2, "w2f")
        wbf2 = sbuf.tile([128, KC, D], BF16, tag="wbf", name="wbf2")
        for kc in range(KC):
            nc.vector.tensor_copy(out=wbf2[:, kc, :], in_=w2_sb[:, kc, :])
        b2f = emit_bias_fold([wbf2[:, kc, :] for kc in range(KC)],
                             t2, bb2c, "b2f")
        # stage w3 (slot WAR clears right here, at the w2 casts)
        w3_sb = wstage.tile([128, KC, D], F32, tag="wst", name="w3_sb")
        for kc in range(KC):
            nc.sync.dma_start(out=w3_sb[:, kc, :],
                              in_=ins["w3"][kc * 128:(kc + 1) * 128, :])
        T2 = sbuf.tile([128, KC, NTILES], F32, tag="T2", name="T2")
        Q2 = sbuf.tile([128, KC, NQSP], F32, tag="Q2", name="Q2")
        for j in range(NTILES):
            ps = layer_matmuls(w2f, j)
            layer_drains(ps, b2f, T2, j)
            emit_q_spans(j, Q2)

        # ================= barrier #3 + L3 + pooling + FC head ===============
        with tc.tile_pool(name="fcpool", bufs=1) as fcpool:
            arm_sb = fcpool.tile([NTILES, G], F32, tag="arm", name="arm_sb")
            qconst.dma_start(out=arm_sb, in_=ins["arm"])
            oh_sb = fcpool.tile([125, KMIX, KC, G], BF16, tag="oh", name="oh_sb")
            qconst.dma_start(out=oh_sb, in_=ins["oh"])
            ident = fcpool.tile([128, 128], F32, tag="ident", name="ident")
            from concourse.masks import make_identity
            make_identity(nc, ident)
            identG = fcpool.tile([G, G], F32, tag="identG", name="identG")
            make_identity(nc, identG)
            identb = fcpool.tile([128, 128], BF16, tag="identb", name="identb")
            nc.vector.tensor_copy(out=identb, in_=ident)
            ngb = fcpool.tile([128, G], F32, tag="ngb", name="ngb")
            qconst.dma_start(out=ngb, in_=ins["cnts"].to_broadcast([128, G]))
            a3b = fcpool.tile([128, 1], F32, tag="a3b", name="a3b")
            qconst.dma_start(out=a3b, in_=ins["a3v"].to_broadcast([128, 1]))
            fc1bc_sb = fcpool.tile([128, KC], F32, tag="fc1bc", name="fc1bc_sb")
            qconst.dma_start(out=fc1bc_sb, in_=ins["fc1bc"])
            fc2bc_sb = fcpool.tile([128, 2], F32, tag="fc2bc", name="fc2bc_sb")
            qconst.dma_start(out=fc2bc_sb, in_=ins["fc2bc"])
            fc3bc_sb = fcpool.tile([C, 1], F32, tag="fc3bc", name="fc3bc_sb")
            qconst.dma_start(out=fc3bc_sb, in_=ins["fc3bc"])

            red3 = stats_allreduce(T2, Q2, KC)
            s3, t3 = emit_s_t(red3, KC, bn3g, bn3b)
            w3f = emit_w_cast(w3_sb, s3, "w3f")
            wbf3 = sbuf.tile([128, KC, D], BF16, tag="wbf", name="wbf3")
            for kc in range(KC):
                nc.vector.tensor_copy(out=wbf3[:, kc, :], in_=w3_sb[:, kc, :])
            b3f = emit_bias_fold([wbf3[:, kc, :] for kc in range(KC)],
                                 t3, bb3c, "b3f")
            # stage fc1w (slot WAR clears at the w3 casts); bf16 cast follows
            fc1w_sb = wstage.tile([128, KC, D], F32, tag="wst", name="fc1w_sb")
            for kc in range(KC):
                nc.sync.dma_start(out=fc1w_sb[:, kc, :],
                                  in_=ins["fc1w"][kc * 128:(kc + 1) * 128, :])

            # ---- L3: r3 -> R in place; per-tile sums + sumsq ----------------
            TC = sbuf.tile([128, KC, NTILES], F32, tag="TC", name="TC")
            Q3 = sbuf.tile([128, KC, NQSP], F32, tag="Q3", name="Q3")
            for j in range(NTILES):
                ps = layer_matmuls(w3f, j)
                layer_drains(ps, b3f, TC, j)
                emit_q_spans(j, Q3)

            # bf16 copies of the fc weights (DMA'd during L3)
            fc1wb = fcpool.tile([128, KC, D], BF16, tag="fc1wb", name="fc1wb")
            for kc in range(KC):
                nc.vector.tensor_copy(out=fc1wb[:, kc, :], in_=fc1w_sb[:, kc, :])
            fc2wb = fcpool.tile([128, KC, 256], BF16, tag="fc2wb", name="fc2wb")
            for kc in range(KC):
                nc.vector.tensor_copy(out=fc2wb[:, kc, :], in_=fc2w_sb[:, kc, :])
            fc3wb = fcpool.tile([128, 2, C], BF16, tag="fc3wb", name="fc3wb")
            for kc in range(2):
                nc.vector.tensor_copy(out=fc3wb[:, kc, :], in_=fc3w_sb[:, kc, :])

            # pooled partials: arm.T @ TC.T (pure tiles, fp32) plus the
            # node-level one-hot matmul for the K mixed tiles (bf16 r3).
            poolG = fcpool.tile([G, KC, 128], F32, tag="poolG", name="poolG")
            mix0 = (NTILES - KMIX) * NT
            for dc in range(KC):
                pT = psum.tile([NTILES, 128], F32, tag="ps", name="pT")
                nc.tensor.transpose(pT, TC[:, dc, :], ident)
                tct = scratch.tile([NTILES, 128], F32, tag="tct",
                                   name="tct")
                nc.vector.tensor_copy(out=tct, in_=pT)
                pG = psum.tile([G, 128], F32, tag="ps", name="pG")
                nc.tensor.matmul(pG, lhsT=arm_sb, rhs=tct, start=True, stop=True)
                # mixed tiles: transpose r3 [128,125] pieces, one-hot matmul
                pM = psum.tile([G, 128], F32, tag="ps", name="pM")
                nmm = KMIX * KC
                i = 0
                for k in range(KMIX):
                    for pc in range(KC):
                        a = mix0 + k * NT + pc * 125
                        pR = psum.tile([125, 128], BF16, tag="ps", name="pR")
                        nc.tensor.transpose(pR, R[:, dc, a:a + 125], identb)
                        r3t = scratch.tile([125, 128], BF16, tag="r3t",
                                           name="r3t")
                        nc.vector.tensor_copy(out=r3t, in_=pR)
                        nc.tensor.matmul(pM, lhsT=oh_sb[:, k, pc, :], rhs=r3t,
                                         start=(i == 0), stop=(i == nmm - 1))
                        i += 1
                pMs = scratch.tile([G, 128], F32, tag="pms", name="pMs")
                nc.vector.tensor_copy(out=pMs, in_=pM)
                nc.vector.tensor_tensor(out=poolG[:, dc, :], in0=pG, in1=pMs,
                                        op=ADD)

            # pack bn3#3 stats + pooled partials into one AllReduce
            nst = 128 * KC * 2
            flat = dram.tile([nst + G * KC * 128], F32, tag="cc4in", name="flat")
            flat_out = dram.tile([nst + G * KC * 128], F32, tag="cc4out",
                                 name="flat_out")
            pack = sbuf.tile([128, KC, 2], F32, tag="statpack", name="pack4")
            nc.vector.tensor_reduce(out=pack[:, :, 0], in_=TC,
                                    axis=AXX, op=ADD)
            nc.vector.tensor_reduce(out=pack[:, :, 1], in_=Q3, axis=AXX, op=ADD)
            red4 = sbuf.tile([128, KC, 2], F32, tag="statred", name="red4")
            poolGr = fcpool.tile([G, KC, 128], F32, tag="poolGr", name="poolGr")
            nc.gpsimd.dma_start(
                out=flat[0:nst].rearrange("(p c k) -> p c k", p=128, c=KC),
                in_=pack)
            nc.gpsimd.dma_start(
                out=flat[nst:].rearrange("(g c f) -> g c f", g=G, c=KC),
                in_=poolG)
            nc.gpsimd.collective_compute(
                "AllReduce", ADD, replica_groups=AR_GROUPS,
                ins=[flat.opt()], outs=[flat_out.opt()])
            nc.gpsimd.dma_start(
                out=red4,
                in_=flat_out[0:nst].rearrange("(p c k) -> p c k", p=128, c=KC))
            nc.gpsimd.dma_start(
                out=poolGr,
                in_=flat_out[nst:].rearrange("(g c f) -> g c f", g=G, c=KC))

            s4, t4 = emit_s_t(red4, KC, bn3g, bn3b)

            # pooled_bn[f, g] = s4[f]*pooled[f, g] + t4[f]*n[g] (feature-major,
            # bf16 for the fc matmuls)
            pooled = fcpool.tile([128, KC, G], BF16, tag="pooled", name="pooled")
            for dc in range(KC):
                pF = psum.tile([128, G], F32, tag="ps", name="pF")
                nc.tensor.transpose(pF, poolGr[:, dc, :], identG)
                tmp = scratch.tile([128, G], F32, tag="ngt", name="tmp")
                nc.vector.tensor_scalar(
                    out=tmp, in0=pF, scalar1=s4[:, dc:dc + 1],
                    scalar2=None, op0=MULT)
                tmp2 = scratch.tile([128, G], F32, tag="ngt2", name="tmp2")
                nc.vector.tensor_scalar(
                    out=tmp2, in0=ngb, scalar1=t4[:, dc:dc + 1],
                    scalar2=None, op0=MULT)
                nc.vector.tensor_add(pooled[:, dc, :], tmp, tmp2)

            # ---------------- FC head (bf16 matmuls) -------------------------
            h1 = fcpool.tile([128, KC, G], BF16, tag="h1", name="h1")
            for dc in range(KC):
                p = psum.tile([128, G], F32, tag="ps", name="pfc")
                for kc in range(KC):
                    nc.tensor.matmul(
                        p, lhsT=fc1wb[:, kc, dc * 128:(dc + 1) * 128],
                        rhs=pooled[:, kc, :], start=(kc == 0), stop=(kc == KC - 1))
                z = scratch.tile([128, G], F32, tag="fcz", name="z")
                nc.vector.tensor_scalar(out=z, in0=p,
                                        scalar1=fc1bc_sb[:, dc:dc + 1],
                                        scalar2=None, op0=ADD)
                # prelu(z) = max(a*z, z) for 0 <= a <= 1
                nc.vector.scalar_tensor_tensor(
                    out=h1[:, dc, :], in0=z, scalar=a3b[:, 0:1], in1=z,
                    op0=MULT, op1=MAX)

            # fc2 + sigmoid -> h2 [128, 2, G]
            h2 = fcpool.tile([128, 2, G], BF16, tag="h2", name="h2")
            for ec in range(2):
                p = psum.tile([128, G], F32, tag="ps", name="pfc2")
                for kc in range(KC):
                    nc.tensor.matmul(
                        p, lhsT=fc2wb[:, kc, ec * 128:(ec + 1) * 128],
                        rhs=h1[:, kc, :], start=(kc == 0), stop=(kc == KC - 1))
                nc.scalar.activation(out=h2[:, ec, :], in_=p, func=Sig,
                                     bias=fc2bc_sb[:, ec:ec + 1], scale=1.0)

            # fc3 -> out [10, 64]
            p = psum.tile([C, G], F32, tag="ps", name="pfc3")
            for kc in range(2):
                nc.tensor.matmul(p, lhsT=fc3wb[:, kc, :], rhs=h2[:, kc, :],
                                 start=(kc == 0), stop=(kc == 1))
            ob = fcpool.tile([C, G], F32, tag="ob", name="ob")
            nc.vector.tensor_scalar(out=ob, in0=p, scalar1=fc3bc_sb,
                                    scalar2=None, op0=ADD)
            nc.sync.dma_start(out=out_ap, in_=ob)


_cached = {}


def kernel(**inputs) -> np.ndarray:
    in_maps, kmix = _build_host_inputs(inputs)
    if _cached.get("kmix") != kmix:
        _cached["nc"] = build_program(kmix)
        _cached["kmix"] = kmix
    nc = _cached["nc"]
    res = bass_utils.run_bass_kernel_spmd(
        nc, in_maps, core_ids=list(range(NCORES)))
    out = res.results[0]["out"]  # [10, 64]
    return np.ascontiguousarray(out.T.astype(np.float32))


if __name__ == "__main__":
    import reference
    inp = {k: np.asarray(v) for k, v in reference.setup_inputs().items()}
    got = kernel(**inp)
    exp = np.asarray(reference.reference(**{
        k: np.asarray(v) for k, v in reference.setup_inputs().items()}))
    err = np.linalg.norm(got - exp) / np.linalg.norm(exp)
    print("Relative error:", err)

